# revision 26
# baseline (speedup 1.0000x reference)
"""BiDirectionalAttention (BiDAF-style) Trainium2 Bass kernel.

Full-input contract: kernel(**inputs) takes the complete unsharded inputs and
returns the full [32, 2048, 512] output. Internally the work is data-parallel
over batch: 8 NeuronCores x 4 batches each.

Per batch b (C=2048 context rows, Q=64 question rows, H=128):
  sim[c,q] = <ctx[c]*w_m, qst[q]> + <w_c, ctx[c]> + <w_q, qst[q]> + mask
  q2c      = softmax_q(sim) @ qst
  c2q      = softmax_c(max_q sim) @ ctx          (one H-vector per batch)
  out      = [ctx | q2c | ctx*q2c | ctx*c2q]     (ctx block assembled on host)

Layout/scheduling choices (HW-measured: the kernel is latency/op-count bound,
so work is batched into 8-tile waves = one full PSUM bank per step):
  - context supplied twice: natural [C,H] and pre-transposed [H,C] (the sim
    matmul contracts over H, which must sit on the partition dim for the PE).
  - sim for a wave of 8 c-tiles fills one PSUM bank [128, 8x64] exactly; the
    <w_c, ctx> column goes to a separate persistent bank via an N=1 matmul
    that reuses the already-loaded ctxT weights.
  - the question bias row (w_q dot + mask) is added by one K=1 ones-matmul
    per wave; softmax-q stats run on the whole wave with a shared shift
    (exact softmax: shift invariance; exp stays in (0,1]).
  - q2c needs e transposed: PE transpose per tile, one ACT copy per wave.
  - softmax over c uses a PE transpose of row maxima + ones-matmul reductions.
  - outputs ship per wave: cols 128:384 right after phase 1, 384:512 after
    the c2q phase.
"""

from contextlib import ExitStack

import numpy as np

import concourse.bacc as bacc
import concourse.bass as bass
import concourse.mybir as mybir
import concourse.tile as tile
from concourse.bass import ts
from concourse.bass_utils import run_bass_kernel_spmd

F32 = mybir.dt.float32
AX = mybir.AxisListType
OP = mybir.AluOpType
AF = mybir.ActivationFunctionType

B, C, Q, H = 32, 2048, 64, 128
NEG = -1e9
NCORES = 8
BP = B // NCORES      # batches per core
TP = 128              # c rows per tile (partition dim)
NT = C // TP          # 16 tiles per batch
WT = 8                # tiles per wave (8 x [128,64] sim = one PSUM bank)
NW = NT // WT


def _flat2(t3, total):
    """2D contiguous view of a [P, a, b] tile with dense free dims."""
    return bass.AP(tensor=t3.tensor, offset=t3.offset, ap=[t3.ap[0], [1, total]])


def build_module(sim_safe=False, repeat=None):
    # sim_safe: CoreSim's matmul visitor compares result vs out_view shapes
    # without flattening; per-tile bias matmuls avoid its 3D-strided case.
    # repeat: wrap the workload in a hardware For_i loop (benchmark only).
    nc = bacc.Bacc("TRN2", debug=False, num_devices=NCORES)

    ctx_nat = nc.dram_tensor("ctx_nat", [BP, C, H], F32, kind="ExternalInput")
    ctx_t = nc.dram_tensor("ctx_t", [BP, H, C], F32, kind="ExternalInput")
    qst = nc.dram_tensor("qst", [BP, Q, H], F32, kind="ExternalInput")
    rhs_aug = nc.dram_tensor("rhs_aug", [BP, H, Q + 1], F32, kind="ExternalInput")
    bias8 = nc.dram_tensor("bias8", [BP, 1, WT * Q], F32, kind="ExternalInput")
    ident = nc.dram_tensor("ident", [H, H], F32, kind="ExternalInput")
    out = nc.dram_tensor("out", [BP, C, 3 * H], F32, kind="ExternalOutput")

    ctx_nat_ap = ctx_nat.ap().rearrange("b (t p) h -> b p t h", p=TP)
    ctx_t_ap = ctx_t.ap()
    qst_ap = qst.ap()
    rhs_aug_ap = rhs_aug.ap()
    bias8_ap = bias8.ap()
    out_full = out.ap().rearrange("b (t p) j -> b p t j", p=TP)
    out12_ap = out_full[:, :, :, 0 : 2 * H]
    out4_ap = out_full[:, :, :, 2 * H : 3 * H]

    with tile.TileContext(nc) as tc, ExitStack() as ctx:
        const = ctx.enter_context(tc.tile_pool(name="const", bufs=1))
        big = ctx.enter_context(tc.tile_pool(name="big", bufs=2))
        med = ctx.enter_context(tc.tile_pool(name="med", bufs=3))
        small = ctx.enter_context(tc.tile_pool(name="small", bufs=2))
        outp = ctx.enter_context(tc.tile_pool(name="outp", bufs=2))
        ps_sim = ctx.enter_context(tc.tile_pool(name="ps_sim", bufs=2, space="PSUM"))
        ps_et = ctx.enter_context(tc.tile_pool(name="ps_et", bufs=1, space="PSUM"))
        ps_q2c = ctx.enter_context(tc.tile_pool(name="ps_q2c", bufs=1, space="PSUM"))
        ps_cwc = ctx.enter_context(tc.tile_pool(name="ps_cwc", bufs=1, space="PSUM"))
        ps_misc = ctx.enter_context(tc.tile_pool(name="ps_misc", bufs=1, space="PSUM"))

        ident_sb = const.tile([H, H], F32)
        nc.sync.dma_start(out=ident_sb, in_=ident.ap())
        ones_row = const.tile([1, H], F32)
        nc.vector.memset(ones_row, 1.0)
        ones_col = const.tile([H, 1], F32)
        nc.vector.memset(ones_col, 1.0)

        rep_ctx = tc.For_i(0, repeat, 1) if repeat else None
        if rep_ctx is not None:
            rep_ctx.__enter__()
        for b in range(BP):
            ctxn_sb = big.tile([TP, NT, H], F32, tag="ctxn")
            ctxt_sb = big.tile([H, C], F32, tag="ctxt")
            for w in range(NW):
                nc.sync.dma_start(
                    out=ctxt_sb[:, w * WT * TP : (w + 1) * WT * TP],
                    in_=ctx_t_ap[b][:, w * WT * TP : (w + 1) * WT * TP],
                )
                nc.sync.dma_start(
                    out=ctxn_sb[:, w * WT : (w + 1) * WT, :],
                    in_=ctx_nat_ap[b][:, w * WT : (w + 1) * WT, :],
                )
            qst_sb = med.tile([Q, H], F32, tag="qst")
            nc.sync.dma_start(out=qst_sb, in_=qst_ap[b])
            rhsA_sb = med.tile([H, Q + 1], F32, tag="rhs")
            nc.sync.dma_start(out=rhsA_sb, in_=rhs_aug_ap[b])
            bias_sb = med.tile([1, WT * Q], F32, tag="bias")
            nc.sync.dma_start(out=bias_sb, in_=bias8_ap[b])

            negm = small.tile([TP, NT], F32, tag="negm")
            ssum = small.tile([TP, NT], F32, tag="ssum")
            rall = small.tile([TP, NT], F32, tag="rall")
            rmal = small.tile([TP, NT], F32, tag="rmal")
            stage = outp.tile([TP, NT, 2 * H], F32, tag="stage12")
            stage4 = outp.tile([TP, NT, H], F32, tag="stage4")
            cwc_ps = ps_cwc.tile([TP, NT], F32, tag="cwc")

            # ---------------- phase 1: sim -> softmax_q -> q2c, per wave ----
            for w in range(NW):
                wsl = slice(w * WT, (w + 1) * WT)
                sim = ps_sim.tile([TP, WT, Q], F32, tag="sim")
                sim2d = _flat2(sim, WT * Q)
                for k in range(WT):
                    t = w * WT + k
                    nc.tensor.matmul(
                        sim[:, k, :],
                        lhsT=ctxt_sb[:, ts(t, TP)],
                        rhs=rhsA_sb[:, 0:Q],
                        start=(k == 0),
                        stop=False,
                    )
                    # <w_c, ctx> column: same stationary weights, N=1
                    nc.tensor.matmul(
                        cwc_ps[:, t : t + 1],
                        lhsT=ctxt_sb[:, ts(t, TP)],
                        rhs=rhsA_sb[:, Q : Q + 1],
                        start=(t == 0),
                        stop=(t == NT - 1),
                    )
                # bias row broadcast into the whole wave (K=1 rank-1 update)
                if sim_safe:
                    for k in range(WT):
                        nc.tensor.matmul(
                            sim[:, k, :],
                            lhsT=ones_row,
                            rhs=bias_sb[:, 0:Q],
                            start=False,
                            stop=(k == WT - 1),
                        )
                else:
                    nc.tensor.matmul(
                        sim2d,
                        lhsT=ones_row,
                        rhs=bias_sb,
                        start=False,
                        stop=True,
                    )

                nc.vector.tensor_reduce(
                    out=negm[:, wsl], in_=sim, axis=AX.X, op=OP.max, negate=True
                )
                # shared shift for the wave (softmax is shift invariant;
                # per-row max <= wave max keeps exp in (0, 1])
                negm_sh = small.tile([TP, 1], F32, tag="negmsh")
                nc.vector.tensor_reduce(
                    out=negm_sh, in_=negm[:, wsl], axis=AX.X, op=OP.min
                )
                e_sb = med.tile([TP, WT, Q], F32, tag="e")
                nc.scalar.activation(
                    out=_flat2(e_sb, WT * Q),
                    in_=sim2d,
                    func=AF.Exp,
                    bias=negm_sh,
                    scale=1.0,
                )
                nc.vector.tensor_reduce(
                    out=ssum[:, wsl], in_=e_sb, axis=AX.X, op=OP.add
                )
                nc.vector.reciprocal(rall[:, wsl], ssum[:, wsl])
                rall_b = bass.AP(
                    tensor=rall.tensor,
                    offset=rall[:, wsl].offset,
                    ap=[rall.ap[0], [rall.ap[1][0], WT], [0, Q]],
                )
                nc.vector.tensor_mul(e_sb, e_sb, rall_b)
                eT_ps = ps_et.tile([Q, WT, TP], F32, tag="eT")
                for k in range(WT):
                    nc.tensor.matmul(
                        eT_ps[:, k, :],
                        lhsT=e_sb[:, k, :],
                        rhs=ident_sb,
                        is_transpose=True,
                        start=(k % 4 == 0),
                        stop=(k % 4 == 3),
                    )
                eT_sb = med.tile([Q, WT, TP], F32, tag="eTs")
                nc.scalar.copy(
                    out=_flat2(eT_sb, WT * TP), in_=_flat2(eT_ps, WT * TP)
                )
                q2c_ps = ps_q2c.tile([TP, WT, H], F32, tag="q2c")
                for k in range(WT):
                    nc.tensor.matmul(
                        q2c_ps[:, k, :],
                        lhsT=eT_sb[:, k, :],
                        rhs=qst_sb,
                        start=(k % 4 == 0),
                        stop=(k % 4 == 3),
                    )
                nc.scalar.copy(out=stage[:, wsl, 0:H], in_=q2c_ps)
                nc.vector.tensor_mul(
                    stage[:, wsl, H : 2 * H], q2c_ps, ctxn_sb[:, wsl, :]
                )
                nc.sync.dma_start(
                    out=out12_ap[b][:, wsl, :], in_=stage[:, wsl, :]
                )

            # ---------------- phase 2: softmax over c, c2q ------------------
            nc.vector.tensor_sub(rmal, cwc_ps, negm)
            mx1 = small.tile([TP, 1], F32, tag="mx1")
            nc.vector.tensor_reduce(out=mx1, in_=rmal, axis=AX.X, op=OP.max)
            mxT_ps = ps_misc.tile([1, TP], F32, tag="ph2s")
            nc.tensor.transpose(mxT_ps, mx1, ident_sb)
            mxT_sb = small.tile([1, TP], F32, tag="mxT")
            nc.vector.tensor_scalar_mul(mxT_sb, mxT_ps, -1.0)
            negM1 = small.tile([1, 1], F32, tag="negM1")
            nc.vector.tensor_reduce(out=negM1, in_=mxT_sb, axis=AX.X, op=OP.min)
            negM_ps = ps_misc.tile([TP, 1], F32, tag="ph2s")
            nc.tensor.matmul(negM_ps, lhsT=ones_row, rhs=negM1, start=True, stop=True)
            negMb = small.tile([TP, 1], F32, tag="negMb")
            nc.vector.tensor_copy(out=negMb, in_=negM_ps)
            exp_rm = small.tile([TP, NT], F32, tag="exprm")
            psums = small.tile([TP, 1], F32, tag="psums")
            nc.scalar.activation(
                out=exp_rm,
                in_=rmal,
                func=AF.Exp,
                bias=negMb,
                scale=1.0,
                accum_out=psums,
            )
            s_ps = ps_misc.tile([1, 1], F32, tag="ph2s")
            nc.tensor.matmul(s_ps, lhsT=psums, rhs=ones_col, start=True, stop=True)
            s_r = small.tile([1, 1], F32, tag="s_r")
            nc.vector.reciprocal(s_r, s_ps)
            c2q_ps = ps_misc.tile([1, H], F32, tag="ph2s")
            for t in range(NT):
                nc.tensor.matmul(
                    c2q_ps,
                    lhsT=exp_rm[:, t : t + 1],
                    rhs=ctxn_sb[:, t, :],
                    start=(t == 0),
                    stop=(t == NT - 1),
                )
            c2q_sb = small.tile([1, H], F32, tag="c2q")
            nc.vector.tensor_scalar_mul(c2q_sb, c2q_ps, s_r)
            c2qb_ps = ps_misc.tile([H, H], F32, tag="ph2s")
            nc.tensor.matmul(c2qb_ps, lhsT=ones_row, rhs=c2q_sb, start=True, stop=True)
            c2qb_sb = small.tile([H, H], F32, tag="c2qb")
            nc.scalar.copy(out=c2qb_sb, in_=c2qb_ps)

            # ---------------- phase 3: ctx * c2q elementwise on DVE ---------
            c2qb_b = bass.AP(
                tensor=c2qb_sb.tensor,
                offset=c2qb_sb.offset,
                ap=[c2qb_sb.ap[0], [0, WT], c2qb_sb.ap[1]],
            )
            for w in range(NW):
                wsl = slice(w * WT, (w + 1) * WT)
                nc.vector.tensor_mul(
                    stage4[:, wsl, :], ctxn_sb[:, wsl, :], c2qb_b
                )
                nc.sync.dma_start(
                    out=out4_ap[b][:, wsl, :], in_=stage4[:, wsl, :]
                )
        if rep_ctx is not None:
            rep_ctx.__exit__(None, None, None)

    nc.compile()
    return nc


_MODULE = None


def _get_module():
    global _MODULE
    if _MODULE is None:
        _MODULE = build_module()
    return _MODULE


def make_in_maps(context, question, question_mask, att_weight):
    """Host-side prep: sharding + layout transforms (no O(B*C*Q*H) compute)."""
    context = np.ascontiguousarray(np.asarray(context, np.float32))
    question = np.ascontiguousarray(np.asarray(question, np.float32))
    qmask = np.asarray(question_mask)
    att_weight = np.asarray(att_weight, np.float32)
    w_c, w_q, w_m = att_weight[:H], att_weight[H : 2 * H], att_weight[2 * H :]

    ctx_t = np.ascontiguousarray(context.transpose(0, 2, 1))
    qmw_t = np.ascontiguousarray((question * w_m[None, None, :]).transpose(0, 2, 1))
    rhs_aug = np.concatenate(
        [qmw_t, np.broadcast_to(w_c[None, :, None], (B, H, 1))], axis=2
    ).astype(np.float32)
    bias = (question @ w_q).astype(np.float32) + np.where(
        qmask, np.float32(0.0), np.float32(NEG)
    ).astype(np.float32)
    bias8 = np.ascontiguousarray(
        np.tile(bias, (1, WT)).reshape(B, 1, WT * Q).astype(np.float32)
    )
    ident = np.eye(H, dtype=np.float32)

    in_maps = []
    for i in range(NCORES):
        sl = slice(i * BP, (i + 1) * BP)
        in_maps.append(
            {
                "ctx_nat": np.ascontiguousarray(context[sl]),
                "ctx_t": np.ascontiguousarray(ctx_t[sl]),
                "qst": np.ascontiguousarray(question[sl]),
                "rhs_aug": np.ascontiguousarray(rhs_aug[sl]),
                "bias8": np.ascontiguousarray(bias8[sl]),
                "ident": ident,
            }
        )
    return in_maps


def assemble_output(context, core_results):
    out = np.empty((B, C, 4 * H), np.float32)
    out[:, :, :H] = context
    for i, res in enumerate(core_results):
        out[i * BP : (i + 1) * BP, :, H:] = res["out"]
    return out


def run(inputs, trace=False, **kwargs):
    context = np.asarray(inputs["context"], np.float32)
    in_maps = make_in_maps(
        context,
        inputs["question"],
        inputs["question_mask"],
        inputs["att_weight"],
    )
    nc = _get_module()
    res = run_bass_kernel_spmd(
        nc, in_maps, core_ids=list(range(NCORES)), trace=trace, **kwargs
    )
    return assemble_output(context, res.results), res


def kernel(**inputs):
    out, _ = run(inputs, trace=False)
    return out


# revision 27
# speedup vs baseline: 1.1719x; 1.1719x over previous
"""BiDirectionalAttention (BiDAF-style) Trainium2 Bass kernel.

Full-input contract: kernel(**inputs) takes the complete unsharded inputs and
returns the full [32, 2048, 512] output. Internally the work is data-parallel
over batch: 8 NeuronCores x 4 batches each.

Per batch b (C=2048 context rows, Q=64 question rows, H=128):
  sim[c,q] = <ctx[c]*w_m, qst[q]> + <w_c, ctx[c]> + <w_q, qst[q]> + mask
  q2c      = softmax_q(sim) @ qst
  c2q      = softmax_c(max_q sim) @ ctx          (one H-vector per batch)
  out      = [ctx | q2c | ctx*q2c | ctx*c2q]     (ctx block assembled on host)

Device layout choices:
  - context is supplied twice: natural [C,H] (elementwise/c2q/output) and
    pre-transposed [H,C] (the sim matmul contracts over H, which must sit on
    the partition dim for the PE).
  - sim is built per 128-row c-tile as PSUM [128, 65]: col 64 carries
    <w_c, ctx[c]> for the second softmax; a K=1 ones-matmul adds the
    question bias row (w_q dot + question_mask) across all partitions.
  - softmax over q is free-dim; softmax over c uses a PE transpose of the
    per-row maxima + a ones-matmul partition reduction.
  - ctx*c2q is computed on the PE as ctxT_tile.T @ diag(c2q).
"""

import os
from contextlib import ExitStack

import numpy as np

import concourse.bacc as bacc
import concourse.mybir as mybir
import concourse.tile as tile
import concourse.bass as bass
from concourse.bass import ts
from concourse.bass_utils import run_bass_kernel_spmd

F32 = mybir.dt.float32
AX = mybir.AxisListType
OP = mybir.AluOpType
AF = mybir.ActivationFunctionType

B, C, Q, H = 32, 2048, 64, 128
NEG = -1e9
NCORES = 8
BP = B // NCORES      # batches per core
TP = 128              # c rows per tile (partition dim)
NT = C // TP          # 16 tiles per batch
WT = 4                # tiles per wave (4 x [128,65] sim fits one PSUM bank)
NW = NT // WT


def build_module(sim_safe=False, repeat=None):
    # sim_safe: CoreSim's matmul visitor asserts result.shape == out_view.shape
    # without flattening free dims, so the wave-wide bias matmul (3D strided
    # out) trips it. The per-tile variant is numerically identical.
    # repeat: wrap the whole workload in a hardware For_i loop (benchmarking
    # only - reruns the same data; output unchanged).
    nc = bacc.Bacc("TRN2", debug=False, num_devices=NCORES)

    ctx_nat = nc.dram_tensor("ctx_nat", [BP, C, H], F32, kind="ExternalInput")
    ctx_t = nc.dram_tensor("ctx_t", [BP, H, C], F32, kind="ExternalInput")
    qst = nc.dram_tensor("qst", [BP, Q, H], F32, kind="ExternalInput")
    rhs_aug = nc.dram_tensor("rhs_aug", [BP, H, Q + 1], F32, kind="ExternalInput")
    bias4 = nc.dram_tensor("bias4", [BP, 1, WT * Q], F32, kind="ExternalInput")
    ident = nc.dram_tensor("ident", [H, H], F32, kind="ExternalInput")
    out = nc.dram_tensor("out", [BP, C, 3 * H], F32, kind="ExternalOutput")

    ctx_nat_ap = ctx_nat.ap().rearrange("b (t p) h -> b p t h", p=TP)
    ctx_t_ap = ctx_t.ap()
    qst_ap = qst.ap()
    rhs_aug_ap = rhs_aug.ap()
    bias4_ap = bias4.ap()
    out_full = out.ap().rearrange("b (t p) j -> b p t j", p=TP)
    out12_ap = out_full[:, :, :, 0 : 2 * H]
    out4_ap = out_full[:, :, :, 2 * H : 3 * H]

    with tile.TileContext(nc) as tc, ExitStack() as ctx:
        const = ctx.enter_context(tc.tile_pool(name="const", bufs=1))
        big = ctx.enter_context(tc.tile_pool(name="big", bufs=2))
        med = ctx.enter_context(tc.tile_pool(name="med", bufs=3))
        small = ctx.enter_context(tc.tile_pool(name="small", bufs=2))
        outp = ctx.enter_context(tc.tile_pool(name="outp", bufs=2))
        ps_sim = ctx.enter_context(tc.tile_pool(name="ps_sim", bufs=4, space="PSUM"))
        ps_et = ctx.enter_context(tc.tile_pool(name="ps_et", bufs=1, space="PSUM"))
        ps_q2c = ctx.enter_context(tc.tile_pool(name="ps_q2c", bufs=2, space="PSUM"))
        ps_misc = ctx.enter_context(tc.tile_pool(name="ps_misc", bufs=1, space="PSUM"))

        ident_sb = const.tile([H, H], F32)
        nc.sync.dma_start(out=ident_sb, in_=ident.ap())
        ones_row = const.tile([1, H], F32)
        nc.vector.memset(ones_row, 1.0)
        ones_col = const.tile([H, 1], F32)
        nc.vector.memset(ones_col, 1.0)

        rep_ctx = tc.For_i(0, repeat, 1) if repeat else None
        if rep_ctx is not None:
            rep_ctx.__enter__()
        for b in range(BP):
            ctxn_sb = big.tile([TP, NT, H], F32, tag="ctxn")
            ctxt_sb = big.tile([H, C], F32, tag="ctxt")
            for w in range(NW):
                nc.sync.dma_start(
                    out=ctxt_sb[:, w * WT * TP : (w + 1) * WT * TP],
                    in_=ctx_t_ap[b][:, w * WT * TP : (w + 1) * WT * TP],
                )
                nc.sync.dma_start(
                    out=ctxn_sb[:, w * WT : (w + 1) * WT, :],
                    in_=ctx_nat_ap[b][:, w * WT : (w + 1) * WT, :],
                )
            qst_sb = med.tile([Q, H], F32, tag="qst")
            nc.sync.dma_start(out=qst_sb, in_=qst_ap[b])
            rhsA_sb = med.tile([H, Q + 1], F32, tag="rhs")
            nc.sync.dma_start(out=rhsA_sb, in_=rhs_aug_ap[b])
            bias_sb = med.tile([1, WT * Q], F32, tag="bias")
            nc.sync.dma_start(out=bias_sb, in_=bias4_ap[b])
            bias_w = bias_sb.rearrange("o (k q) -> o k q", k=WT)

            negm = small.tile([TP, NT], F32, tag="negm")
            ssum = small.tile([TP, NT], F32, tag="ssum")
            rall = small.tile([TP, NT], F32, tag="rall")
            rmal = small.tile([TP, NT], F32, tag="rmal")
            stage = outp.tile([TP, NT, 2 * H], F32, tag="stage12")
            stage4 = outp.tile([TP, NT, H], F32, tag="stage4")

            # ---------------- phase 1: sim -> softmax_q -> q2c, per wave ----
            for w in range(NW):
                wsl = slice(w * WT, (w + 1) * WT)
                # The whole wave's sim shares one PSUM bank: a single chained
                # accumulation group (one start, one stop) keeps every write
                # on the lazily-zeroed path.
                sim = ps_sim.tile([TP, WT, Q + 1], F32, tag="sim")
                # (cwc column kept at index Q per tile)
                for k in range(WT):
                    t = w * WT + k
                    nc.tensor.matmul(
                        sim[:, k, :],
                        lhsT=ctxt_sb[:, ts(t, TP)],
                        rhs=rhsA_sb,
                        start=(k == 0),
                        stop=False,
                    )
                # bias row broadcast into all tiles (K=1 rank-1 update)
                if sim_safe:
                    for k in range(WT):
                        nc.tensor.matmul(
                            sim[:, k, 0:Q],
                            lhsT=ones_row,
                            rhs=bias_w[:, k, :],
                            start=False,
                            stop=(k == WT - 1),
                        )
                else:
                    nc.tensor.matmul(
                        sim[:, :, 0:Q],
                        lhsT=ones_row,
                        rhs=bias_w,
                        start=False,
                        stop=True,
                    )

                nc.vector.tensor_reduce(
                    out=negm[:, wsl],
                    in_=sim[:, :, 0:Q],
                    axis=AX.X,
                    op=OP.max,
                    negate=True,
                )
                # shared shift for the whole wave (softmax is shift invariant;
                # per-row max <= wave max keeps exp in (0, 1])
                negm_sh = small.tile([TP, 1], F32, tag="negmsh")
                nc.vector.tensor_reduce(
                    out=negm_sh, in_=negm[:, wsl], axis=AX.X, op=OP.min
                )
                e_sb = med.tile([TP, WT, Q], F32, tag="e")
                nc.scalar.activation(
                    out=e_sb,
                    in_=sim[:, :, 0:Q],
                    func=AF.Exp,
                    bias=negm_sh,
                    scale=1.0,
                )
                nc.vector.tensor_reduce(
                    out=ssum[:, wsl], in_=e_sb, axis=AX.X, op=OP.add
                )
                # row max for the second softmax: rm = cwc - negm
                nc.vector.tensor_sub(rmal[:, wsl], sim[:, :, Q], negm[:, wsl])
                nc.vector.reciprocal(rall[:, wsl], ssum[:, wsl])
                rall_b = bass.AP(
                    tensor=rall.tensor,
                    offset=rall[:, wsl].offset,
                    ap=[rall.ap[0], [rall.ap[1][0], WT], [0, Q]],
                )
                nc.vector.tensor_mul(e_sb, e_sb, rall_b)
                eT_ps = ps_et.tile([Q, WT, TP], F32, tag="eT")
                for k in range(WT):
                    nc.tensor.matmul(
                        eT_ps[:, k, :],
                        lhsT=e_sb[:, k, :],
                        rhs=ident_sb,
                        is_transpose=True,
                        start=(k == 0),
                        stop=(k == WT - 1),
                    )
                eT_sb = med.tile([Q, WT, TP], F32, tag="eTs")
                nc.scalar.copy(out=eT_sb, in_=eT_ps)
                q2c_ps = ps_q2c.tile([TP, WT, H], F32, tag="q2c")
                for k in range(WT):
                    nc.tensor.matmul(
                        q2c_ps[:, k, :],
                        lhsT=eT_sb[:, k, :],
                        rhs=qst_sb,
                        start=(k == 0),
                        stop=(k == WT - 1),
                    )
                nc.scalar.copy(out=stage[:, wsl, 0:H], in_=q2c_ps)
                nc.vector.tensor_mul(
                    stage[:, wsl, H : 2 * H], q2c_ps, ctxn_sb[:, wsl, :]
                )
                # ship this wave's 256 output columns immediately
                nc.gpsimd.dma_start(
                    out=out12_ap[b][:, wsl, :], in_=stage[:, wsl, :]
                )

            # ---------------- phase 2: softmax over c, c2q ------------------
            mx1 = small.tile([TP, 1], F32, tag="mx1")
            nc.vector.tensor_reduce(out=mx1, in_=rmal, axis=AX.X, op=OP.max)
            # [128,1] -> [1,128] so the global max can be reduced on free dim
            mxT_ps = ps_misc.tile([1, TP], F32, tag="ph2s")
            nc.tensor.transpose(mxT_ps, mx1, ident_sb)
            mxT_sb = small.tile([1, TP], F32, tag="mxT")
            nc.vector.tensor_scalar_mul(mxT_sb, mxT_ps, -1.0)
            negM1 = small.tile([1, 1], F32, tag="negM1")
            nc.vector.tensor_reduce(out=negM1, in_=mxT_sb, axis=AX.X, op=OP.min)
            negM_ps = ps_misc.tile([TP, 1], F32, tag="ph2s")
            nc.tensor.matmul(negM_ps, lhsT=ones_row, rhs=negM1, start=True, stop=True)
            negMb = small.tile([TP, 1], F32, tag="negMb")
            nc.vector.tensor_copy(out=negMb, in_=negM_ps)
            exp_rm = small.tile([TP, NT], F32, tag="exprm")
            psums = small.tile([TP, 1], F32, tag="psums")
            nc.scalar.activation(
                out=exp_rm,
                in_=rmal,
                func=AF.Exp,
                bias=negMb,
                scale=1.0,
                accum_out=psums,
            )
            s_ps = ps_misc.tile([1, 1], F32, tag="ph2s")
            nc.tensor.matmul(s_ps, lhsT=psums, rhs=ones_col, start=True, stop=True)
            s_r = small.tile([1, 1], F32, tag="s_r")
            nc.vector.reciprocal(s_r, s_ps)
            c2q_ps = ps_misc.tile([1, H], F32, tag="ph2s")
            for t in range(NT):
                nc.tensor.matmul(
                    c2q_ps,
                    lhsT=exp_rm[:, t : t + 1],
                    rhs=ctxn_sb[:, t, :],
                    start=(t == 0),
                    stop=(t == NT - 1),
                )
            c2q_sb = small.tile([1, H], F32, tag="c2q")
            nc.vector.tensor_scalar_mul(c2q_sb, c2q_ps, s_r)
            c2qb_ps = ps_misc.tile([H, H], F32, tag="ph2s")
            nc.tensor.matmul(c2qb_ps, lhsT=ones_row, rhs=c2q_sb, start=True, stop=True)
            c2qb_sb = small.tile([H, H], F32, tag="c2qb")
            nc.scalar.copy(out=c2qb_sb, in_=c2qb_ps)

            # ---------------- phase 3: ctx * c2q elementwise on DVE ---------
            for w in range(NW):
                wsl = slice(w * WT, (w + 1) * WT)
                for k in range(WT):
                    t = w * WT + k
                    nc.vector.tensor_mul(
                        stage4[:, t, :], ctxn_sb[:, t, :], c2qb_sb
                    )

            nc.gpsimd.dma_start(out=out4_ap[b], in_=stage4)

    nc.compile()
    return nc


_MODULE = None


def _get_module():
    global _MODULE
    if _MODULE is None:
        _MODULE = build_module()
    return _MODULE


def make_in_maps(context, question, question_mask, att_weight):
    """Host-side prep: sharding + layout transforms (no O(B*C*Q*H) compute)."""
    context = np.ascontiguousarray(np.asarray(context, np.float32))
    question = np.ascontiguousarray(np.asarray(question, np.float32))
    qmask = np.asarray(question_mask)
    att_weight = np.asarray(att_weight, np.float32)
    w_c, w_q, w_m = att_weight[:H], att_weight[H : 2 * H], att_weight[2 * H :]

    ctx_t = np.ascontiguousarray(context.transpose(0, 2, 1))
    qmw_t = np.ascontiguousarray((question * w_m[None, None, :]).transpose(0, 2, 1))
    rhs_aug = np.concatenate(
        [qmw_t, np.broadcast_to(w_c[None, :, None], (B, H, 1))], axis=2
    ).astype(np.float32)
    bias = (question @ w_q).astype(np.float32) + np.where(
        qmask, np.float32(0.0), np.float32(NEG)
    ).astype(np.float32)
    bias4 = np.ascontiguousarray(
        np.tile(bias, (1, WT)).reshape(B, 1, WT * Q).astype(np.float32)
    )
    ident = np.eye(H, dtype=np.float32)

    in_maps = []
    for i in range(NCORES):
        sl = slice(i * BP, (i + 1) * BP)
        in_maps.append(
            {
                "ctx_nat": np.ascontiguousarray(context[sl]),
                "ctx_t": np.ascontiguousarray(ctx_t[sl]),
                "qst": np.ascontiguousarray(question[sl]),
                "rhs_aug": np.ascontiguousarray(rhs_aug[sl]),
                "bias4": np.ascontiguousarray(bias4[sl]),
                "ident": ident,
            }
        )
    return in_maps


def assemble_output(context, core_results):
    out = np.empty((B, C, 4 * H), np.float32)
    out[:, :, :H] = context
    for i, res in enumerate(core_results):
        out[i * BP : (i + 1) * BP, :, H:] = res["out"]
    return out


def run(inputs, trace=False, **kwargs):
    context = np.asarray(inputs["context"], np.float32)
    in_maps = make_in_maps(
        context,
        inputs["question"],
        inputs["question_mask"],
        inputs["att_weight"],
    )
    nc = _get_module()
    res = run_bass_kernel_spmd(
        nc, in_maps, core_ids=list(range(NCORES)), trace=trace, **kwargs
    )
    return assemble_output(context, res.results), res


def kernel(**inputs):
    out, _ = run(inputs, trace=False)
    return out
